# revision 1
# baseline (speedup 1.0000x reference)
"""Trainium2 Bass kernel for a pre-norm transformer block (nn_Block).

Math (per batch b of x [4, 1024, 1024]):
    h  = LN(x) ; qkv = h @ w_qkv + b_qkv ; attention (16 heads, dh=64)
    x  = x + (attn_out @ w_proj + b_proj)
    h  = LN(x) ; x = x + gelu(h @ w_fc1 + b_fc1) @ w_fc2 + b_fc2

Sharding: communication-free hybrid. Core c handles batch b = c // 2 and
query-token half c % 2. Each core computes K and V for its batch's full
1024 tokens (duplicated across the 2 cores sharing a batch, ~14% extra
flops) and everything else only for its own 512 query tokens. No
collectives.

On-chip layout is feature-major ("transposed"): every activation lives as
[features, tokens] so all linear layers run as out^T = W^T @ in^T with the
weight (as stored, [in, out]) the stationary operand and the activation the
moving operand; no transposes are ever materialized. V is produced
token-major directly by swapping the matmul operand roles. Softmax runs on
S^T = K^T-major scores: exp on ACT (scale folded in), denominators come
from an extra ones-column appended to V (row 64 of the PV product), and
normalization is deferred to the [64, 512] attention output. LayerNorm
statistics are computed with ones-vector matmuls (fp32r, full speed) since
features live on partitions; per-token scale/shift vectors are replicated
across partitions with a broadcast DMA and applied on the vector engine.

Matmul operands are bf16 (fp32 accumulation in PSUM); LN stats, softmax,
residuals stay fp32. Measured vs the fp32 reference: rel_l2 ~1.5e-3.
"""

import os
import sys
import types

import numpy as np

# concourse ships in the container; make sure it resolves outside the repo.
try:
    import concourse.bass as bass
except ImportError:  # pragma: no cover
    for _p in ("/opt/trn_rl_repo", "/root/.axon_site/_ro/trn_rl_repo"):
        if os.path.isdir(_p) and _p not in sys.path:
            sys.path.insert(0, _p)
    import concourse.bass as bass

import ml_dtypes
import concourse.tile as tile
import concourse.mybir as mybir
from concourse import bass_utils
from concourse.bass import ds

F32 = mybir.dt.float32
F32R = mybir.dt.float32r
BF16 = mybir.dt.bfloat16
AF = mybir.ActivationFunctionType

C = 1024          # model dim
H = 16            # heads
DH = 64           # head dim
NTOK = 1024       # tokens per batch (keys/values)
NQ = 512          # query tokens per core
KT = C // 128     # 8 feature tiles
HID = 4096
EPS = 1e-5

_cache = {}


# --------------------------------------------------------------------------
# Walrus on this image rejects instructions carrying more than one semaphore
# wait command (the Tile epilogue drain accumulates one per logical
# processor). Split the excess onto dedicated same-engine NOPs.
# --------------------------------------------------------------------------
def _split_wide_waits(nc, max_waits=1):
    ctr = 0
    for f in nc.m.functions:
        for b in f.blocks:
            out, changed = [], False
            for inst in b.instructions:
                si = getattr(inst, "sync_info", None)
                if si is not None and si.on_wait and len(si.on_wait) > max_waits:
                    waits = list(si.on_wait)
                    extra, keep = waits[:-max_waits], waits[-max_waits:]
                    for gs in range(0, len(extra), max_waits):
                        ctr += 1
                        nop = mybir.InstNoOp(
                            name=f"waitsplit-{ctr}", ins=[], outs=[])
                        nop.engine = inst.engine
                        nop.sync_info = mybir.SyncInfo(
                            on_wait=extra[gs:gs + max_waits], on_update=[])
                        out.append(nop)
                    inst.sync_info = mybir.SyncInfo(
                        on_wait=keep, on_update=list(si.on_update))
                    changed = True
                out.append(inst)
            if changed:
                b.instructions = out


def build_program(has_bias, gelu_func=None):
    """Build the single-core SPMD Bass program.

    has_bias: dict of bools for qk/v/proj/fc1/fc2 bias emission.
    gelu_func: override the MLP activation (CoreSim lacks Gelu).
    """
    nc = bass.Bass()

    xT = nc.dram_tensor("xT", [C, NQ], F32, kind="ExternalInput")
    xbT = nc.dram_tensor("xbT", [C, NTOK], BF16, kind="ExternalInput")
    wq_m = nc.dram_tensor("wq_m", [KT, 128, C], BF16, kind="ExternalInput")
    wk_m = nc.dram_tensor("wk_m", [KT, 128, C], BF16, kind="ExternalInput")
    wv_r = nc.dram_tensor("wv_r", [KT, 128, C], BF16, kind="ExternalInput")
    wp_m = nc.dram_tensor("wp_m", [KT, 128, C], BF16, kind="ExternalInput")
    w1_m = nc.dram_tensor("w1_m", [HID // 128, 128, C], BF16, kind="ExternalInput")
    w2_m = nc.dram_tensor("w2_m", [KT, 128, HID], BF16, kind="ExternalInput")
    b_all = nc.dram_tensor("b_all", [1, 3 * C + C + HID + C], BF16,
                           kind="ExternalInput")
    yT = nc.dram_tensor("yT", [C, NQ], F32, kind="ExternalOutput")

    with tile.TileContext(nc) as tc:
        _emit(nc, tc, xT, xbT, wq_m, wk_m, wv_r, wp_m, w1_m, w2_m, b_all,
              yT, has_bias, gelu_func or AF.Gelu)
    return nc


def _emit(nc, tc, xT, xbT, wq_m, wk_m, wv_r, wp_m, w1_m, w2_m, b_all, yT,
          has_bias, gelu_func):
    pers = tc.alloc_tile_pool(name="pers", bufs=1)
    ones_c = pers.tile([128, 1], BF16, tag="ones_c")      # stats lhsT
    nc.vector.memset(ones_c, 1.0)
    ones_r16 = pers.tile([1, NQ], BF16, tag="ones_r16")   # bias rank-1 rhs
    nc.vector.memset(ones_r16, 1.0)
    ones_tok16 = pers.tile([1, 128], BF16, tag="ones_tok16")  # v-bias lhsT
    nc.vector.memset(ones_tok16, 1.0)
    eps_t = pers.tile([128, 1], F32, tag="eps_t")
    nc.vector.memset(eps_t, EPS)

    p_dram = tc.alloc_tile_pool(name="dscratch", bufs=12, space="DRAM")

    def ln_chain(ms, ss, N, pool, nm):
        """From per-chunk sum/sumsq PSUM rows, produce a [128, 2N] SBUF tile
        = [rstd_rep | (mu*rstd)_rep].

        Stays in the [1, N] row the whole way: 1/sqrt(var+eps) is computed
        as Exp(-0.5*Ln(var+eps)) on the scalar engine (the DVE reciprocal
        on a 1-partition row costs ~7 us/KB; Ln+Exp are ~1 us). Only one
        DRAM bounce remains: the partition replication, split over 4 DMA
        queues.
        """
        row = pool.tile([1, 2 * N], F32, tag=f"row_{nm}", name=f"row_{nm}")
        nch = N // 512
        for n in range(nch):
            nc.vector.tensor_copy(row[:, ds(n * 512, 512)], ms[n])
            nc.vector.tensor_copy(row[:, ds(N + n * 512, 512)], ss[n])
        mu = row[:, ds(0, N)]
        es = row[:, ds(N, N)]
        nc.vector.tensor_scalar_mul(mu, mu, 1.0 / C)
        nc.vector.tensor_scalar_mul(es, es, 1.0 / C)
        var = pool.tile([1, N], F32, tag=f"var_{nm}", name=f"var_{nm}")
        nc.vector.tensor_mul(var, mu, mu)
        nc.vector.tensor_sub(var, es, var)
        lnv = pool.tile([1, N], F32, tag=f"lnv_{nm}", name=f"lnv_{nm}")
        nc.scalar.activation(lnv, var, AF.Ln, bias=eps_t[ds(0, 1), :])
        out2 = pool.tile([1, 2 * N], F32, tag=f"out2_{nm}", name=f"out2_{nm}")
        rstd = out2[:, ds(0, N)]
        nc.scalar.activation(rstd, lnv, AF.Exp, scale=-0.5)
        nc.vector.tensor_mul(out2[:, ds(N, N)], mu, rstd)
        drow = p_dram.tile([1, 2 * N], F32, tag="dscratch", name=f"dr_{nm}")
        nc.gpsimd.dma_start(drow, out2)
        rep = pool.tile([128, 2 * N], F32, tag=f"rep_{nm}", name=f"rep_{nm}")
        for q in range(4):
            nc.gpsimd.dma_start(rep[ds(q * 32, 32), :],
                                drow.to_broadcast((32, 2 * N)))
        return rep[:, ds(0, N)], rep[:, ds(N, N)]

    any_bias = any(has_bias.values())
    if any_bias:
        bias_sb = pers.tile([1, 3 * C + C + HID + C], BF16, tag="bias_sb")
        nc.sync.dma_start(bias_sb, b_all[:])
        bq_of, bk_of, bv_of = 0, C, 2 * C
        bp_of, b1_of, b2_of = 3 * C, 4 * C, 4 * C + HID

    # x2 (attention residual) is read until the end; allocate it at the
    # bottom of the pool stack even though it is written only at proj time.
    p_x2 = tc.alloc_tile_pool(name="x2", bufs=KT)

    # ---------------- pools; x loads ------------------------------------
    # Only the bf16 copy of x^T feeds LN1 (stats AND normalize); the fp32
    # query-half x^T is needed first at the proj residual, so its loads are
    # spread through the attention phase.
    p_xT = tc.alloc_tile_pool(name="xT", bufs=KT)
    xt = []

    p_V = tc.alloc_tile_pool(name="V", bufs=KT)
    p_wv = tc.alloc_tile_pool(name="wv", bufs=KT)
    p_h1 = tc.alloc_tile_pool(name="h1", bufs=KT)
    p_xb = tc.alloc_tile_pool(name="xb", bufs=KT)
    p_ln1 = tc.alloc_tile_pool(name="ln1", bufs=1)
    p_sq = tc.alloc_tile_pool(name="sq", bufs=3)
    ps_stat = tc.alloc_tile_pool(name="ps_stat", bufs=1, space="PSUM")

    xbt = []
    for k in range(KT):
        t = p_xb.tile([128, NTOK], BF16, tag="xb")
        nc.sync.dma_start(t, xbT[ds(k * 128, 128), :])
        xbt.append(t)
    # V weights + the ones-column memsets go first so the V matmuls are
    # never DMA-gated when h1 tiles start arriving
    wv = []
    for k in range(KT):
        t = p_wv.tile([128, C], BF16, tag="wv")
        nc.sync.dma_start(t, wv_r[k, :, :])
        wv.append(t)
    V = []
    for t in range(KT):
        vt = p_V.tile([128, H, 65], BF16, tag="V", name=f"V{t}")
        nc.vector.memset(vt[:, :, ds(64, 1)], 1.0)
        V.append(vt)

    # stats matmuls run in bf16 (walrus requires fp32r inputs to be
    # explicitly rounded; bf16 stat error ~1e-4, negligible here)
    ms = [ps_stat.tile([1, 512], F32, tag=f"ms{n}", name=f"ms{n}")
          for n in range(2)]
    ss = [ps_stat.tile([1, 512], F32, tag=f"ss{n}", name=f"ss{n}")
          for n in range(2)]
    for k in range(KT):
        sq = p_sq.tile([128, NTOK], BF16, tag="sq")
        nc.scalar.activation(sq, xbt[k], AF.Square)
        for n in range(2):
            nc.tensor.matmul(ms[n], ones_c, xbt[k][:, ds(n * 512, 512)],
                             start=(k == 0), stop=(k == KT - 1))
            nc.tensor.matmul(ss[n], ones_c, sq[:, ds(n * 512, 512)],
                             start=(k == 0), stop=(k == KT - 1))
    p_sq.release()

    rstd_rep, musc_rep = ln_chain(ms, ss, NTOK, p_ln1, "ln1")
    ps_stat.release()

    p_tmp = tc.alloc_tile_pool(name="tmp", bufs=6)
    h1 = []
    for k in range(KT):
        tmp = p_tmp.tile([128, NTOK], F32, tag="tmp")
        nc.vector.tensor_mul(tmp, xbt[k], rstd_rep)
        h = p_h1.tile([128, NTOK], BF16, tag="h1")
        nc.vector.tensor_sub(h, tmp, musc_rep)
        h1.append(h)
    p_tmp.release()
    p_ln1.release()
    p_xb.release()

    # ---------------- V (token-major, with ones column) ----------------
    ps_v = tc.alloc_tile_pool(name="ps_v", bufs=8, space="PSUM")
    # k-outer: each h1[k] is consumed as soon as the normalize produces it,
    # so the PE ramps while the vector engine is still writing h1 tiles.
    for grp in range(2):
        ts_ = range(grp * 4, grp * 4 + 4)
        psv = {(t, n): ps_v.tile([128, 512], F32, tag="ps_v",
                                 name=f"psv{t}_{n}")
               for t in ts_ for n in range(2)}
        for k in range(KT):
            for t in ts_:
                for n in range(2):
                    nc.tensor.matmul(
                        psv[(t, n)], h1[k][:, ds(t * 128, 128)],
                        wv[k][:, ds(n * 512, 512)],
                        start=(k == 0),
                        stop=(k == KT - 1 and not has_bias["v"]))
        for t in ts_:
            for n in range(2):
                if has_bias["v"]:
                    nc.tensor.matmul(
                        psv[(t, n)], ones_tok16,
                        bias_sb[:, ds(bv_of + n * 512, 512)],
                        start=False, stop=True)
                nc.vector.tensor_copy(
                    V[t][:, ds(n * 8, 8), ds(0, 64)],
                    psv[(t, n)].rearrange("p (h d) -> p h d", d=64))
    ps_v.release()

    # ---------------- attention loop over head pairs ----------------
    # pair t = heads (2t, 2t+1); K^T/Q^T feature tile t holds both heads.
    p_O = tc.alloc_tile_pool(name="O", bufs=KT)
    p_K = tc.alloc_tile_pool(name="K", bufs=KT)
    p_Q = tc.alloc_tile_pool(name="Q", bufs=KT)
    p_P = tc.alloc_tile_pool(name="P", bufs=40)
    p_rq = tc.alloc_tile_pool(name="rq", bufs=4)
    p_rep = tc.alloc_tile_pool(name="rep", bufs=4)
    p_wkq = tc.alloc_tile_pool(name="wkq", bufs=4)
    ps_a = tc.alloc_tile_pool(name="ps_a", bufs=2, space="PSUM")
    ps_s = tc.alloc_tile_pool(name="ps_s", bufs=4, space="PSUM")
    ps_o = tc.alloc_tile_pool(name="ps_o", bufs=2, space="PSUM")

    K_sb, Q_sb, P_sb, O_sb = [], [], {}, []

    def emit_kq(t):
        wkt = p_wkq.tile([128, C], BF16, tag="wkq")
        nc.sync.dma_start(wkt, wk_m[t, :, :])
        kt_sb = p_K.tile([128, NTOK], BF16, tag="K")
        for n in range(2):
            ps = ps_a.tile([128, 512], F32, tag="ps_a")
            for k in range(KT):
                nc.tensor.matmul(
                    ps, wkt[:, ds(k * 128, 128)], h1[k][:, ds(n * 512, 512)],
                    start=(k == 0), stop=(k == KT - 1 and not has_bias["qk"]))
            if has_bias["qk"]:
                nc.tensor.matmul(
                    ps, bias_sb[:, ds(bk_of + t * 128, 128)], ones_r16,
                    start=False, stop=True)
            nc.vector.tensor_copy(kt_sb[:, ds(n * 512, 512)], ps)
        K_sb.append(kt_sb)

        wqt = p_wkq.tile([128, C], BF16, tag="wkq")
        nc.sync.dma_start(wqt, wq_m[t, :, :])
        qt_sb = p_Q.tile([128, NQ], BF16, tag="Q")
        ps = ps_a.tile([128, 512], F32, tag="ps_a")
        for k in range(KT):
            nc.tensor.matmul(
                ps, wqt[:, ds(k * 128, 128)], h1[k][:, ds(0, 512)],
                start=(k == 0), stop=(k == KT - 1 and not has_bias["qk"]))
        if has_bias["qk"]:
            nc.tensor.matmul(
                ps, bias_sb[:, ds(bq_of + t * 128, 128)], ones_r16,
                start=False, stop=True)
        nc.vector.tensor_copy(qt_sb, ps)
        Q_sb.append(qt_sb)

    def emit_st(t):
        # S^T then exp, per 128-key block m; two heads ride the PE array
        # concurrently on row strips [0:64] / [64:128].
        for m in range(KT):
            for h2 in range(2):
                lo = h2 * 64
                ps = ps_s.tile([128, 512], F32, tag="ps_s")
                nc.tensor.matmul(
                    ps,
                    K_sb[t][ds(lo, 64), ds(m * 128, 128)],
                    Q_sb[t][ds(lo, 64), :],
                    start=True, stop=True)
                p = p_P.tile([128, 512], BF16, tag="P")
                nc.scalar.activation(p, ps, AF.Exp, scale=float(DH) ** -0.5)
                P_sb[(t, h2, m)] = p

    def emit_pv(t):
        ot = p_O.tile([128, NQ], BF16, tag="O")
        for h2 in range(2):
            head = 2 * t + h2
            ps = ps_o.tile([128, 512], F32, tag="ps_o")
            for k in range(KT):
                nc.tensor.matmul(
                    ps[ds(0, 65), :],
                    V[k][:, head, :],
                    P_sb[(t, h2, k)],
                    start=(k == 0), stop=(k == KT - 1))
            # denominators: 1/x = Exp(-Ln(x)) on ACT (row stays [1,512]),
            # then one DRAM bounce to replicate across partitions
            denr = p_rq.tile([1, 512], F32, tag="denr")
            nc.vector.tensor_copy(denr, ps[ds(64, 1), :])
            lnd = p_rq.tile([1, 512], F32, tag="lnd")
            nc.scalar.activation(lnd, denr, AF.Ln)
            rcp = p_rq.tile([1, 512], F32, tag="rcp")
            nc.scalar.activation(rcp, lnd, AF.Exp, scale=-1.0)
            drr = p_dram.tile([1, 512], F32, tag="dscratch",
                              name=f"drr{t}_{h2}")
            nc.gpsimd.dma_start(drr, rcp)
            rep = p_rep.tile([64, 512], F32, tag="rep")
            for q in range(2):
                nc.gpsimd.dma_start(rep[ds(q * 32, 32), :],
                                    drr.to_broadcast((32, 512)))
            nc.vector.tensor_mul(ot[ds(h2 * 64, 64), :], ps[ds(0, 64), :], rep)
        O_sb.append(ot)

    for t in range(KT):
        emit_kq(t)
        xq = p_xT.tile([128, NQ], F32, tag="xT", name=f"xq{t}")
        nc.sync.dma_start(xq, xT[ds(t * 128, 128), :])
        xt.append(xq)
        emit_st(t)
        if t >= 1:
            emit_pv(t - 1)
    emit_pv(KT - 1)
    for p in (p_wkq, p_rep, p_rq, p_P, p_Q, p_K):
        p.release()
    for p in (ps_o, ps_s, ps_a):
        p.release()

    # ---------------- proj + residual + LN2 stats ----------------
    p_wp = tc.alloc_tile_pool(name="wp", bufs=3)
    p_sq2 = tc.alloc_tile_pool(name="sq2", bufs=3)
    ps_st2 = tc.alloc_tile_pool(name="ps_st2", bufs=1, space="PSUM")
    ps_p = tc.alloc_tile_pool(name="ps_p", bufs=3, space="PSUM")

    ms2 = ps_st2.tile([1, 512], F32, tag="ms2")
    ss2 = ps_st2.tile([1, 512], F32, tag="ss2")
    x2 = []
    for m in range(KT):
        wpt = p_wp.tile([128, C], BF16, tag="wp")
        nc.sync.dma_start(wpt, wp_m[m, :, :])
        ps = ps_p.tile([128, 512], F32, tag="ps_p")
        for k in range(KT):
            nc.tensor.matmul(
                ps, wpt[:, ds(k * 128, 128)], O_sb[k],
                start=(k == 0), stop=(k == KT - 1 and not has_bias["proj"]))
        if has_bias["proj"]:
            nc.tensor.matmul(ps, bias_sb[:, ds(bp_of + m * 128, 128)],
                             ones_r16, start=False, stop=True)
        xm = p_x2.tile([128, NQ], F32, tag="x2")
        nc.vector.tensor_add(xm, ps, xt[m])
        x2.append(xm)
        xb2 = p_sq2.tile([128, NQ], BF16, tag="xb2")
        nc.vector.tensor_copy(xb2, xm)
        sq = p_sq2.tile([128, NQ], BF16, tag="sq2")
        nc.scalar.activation(sq, xm, AF.Square)
        nc.tensor.matmul(ms2, ones_c, xb2,
                         start=(m == 0), stop=(m == KT - 1))
        nc.tensor.matmul(ss2, ones_c, sq,
                         start=(m == 0), stop=(m == KT - 1))

    # release proj-phase + attention carry-over pools (space reused by MLP)
    for p in (p_sq2, p_wp, p_O, p_h1, p_wv, p_V, p_xT):
        p.release()
    ps_p.release()

    # ---------------- LN2 ----------------
    p_ln2 = tc.alloc_tile_pool(name="ln2", bufs=1)
    rstd2_rep, musc2_rep = ln_chain([ms2], [ss2], NQ, p_ln2, "ln2")
    ps_st2.release()

    p_h2 = tc.alloc_tile_pool(name="h2", bufs=KT)
    p_tmp2 = tc.alloc_tile_pool(name="tmp2", bufs=6)
    h2t = []
    for k in range(KT):
        tmp = p_tmp2.tile([128, NQ], F32, tag="tmp2")
        nc.vector.tensor_mul(tmp, x2[k], rstd2_rep)
        h = p_h2.tile([128, NQ], BF16, tag="h2")
        nc.vector.tensor_sub(h, tmp, musc2_rep)
        h2t.append(h)
    p_tmp2.release()

    # ---------------- MLP ----------------
    p_g = tc.alloc_tile_pool(name="g", bufs=HID // 128)
    p_y = tc.alloc_tile_pool(name="y", bufs=3)
    p_w2 = tc.alloc_tile_pool(name="w2", bufs=3)
    p_w1 = tc.alloc_tile_pool(name="w1", bufs=9)
    ps_m8 = tc.alloc_tile_pool(name="ps_m8", bufs=8, space="PSUM")

    # prefetch the first fc1/fc2 weight slices during LN2/fc1
    w1_tiles, w2_tiles = {}, {}
    for m in range(3):
        w1_tiles[m] = p_w1.tile([128, C], BF16, tag="w1", name=f"w1p{m}")
        nc.sync.dma_start(w1_tiles[m], w1_m[m, :, :])
    for m in range(2):
        w2_tiles[m] = p_w2.tile([128, HID], BF16, tag="w2", name=f"w2p{m}")
        nc.sync.dma_start(w2_tiles[m], w2_m[m, :, :])

    g_sb = []
    MT1 = HID // 128
    # first 8 output tiles k-outer: overlap the h2 normalize
    ps8 = [ps_m8.tile([128, 512], F32, tag="ps8", name=f"ps8_{m}")
           for m in range(8)]
    for m in range(8):
        w1_tiles[m] = w1_tiles.get(m)
        if w1_tiles[m] is None:
            w1_tiles[m] = p_w1.tile([128, C], BF16, tag="w1", name=f"w1g{m}")
            nc.sync.dma_start(w1_tiles[m], w1_m[m, :, :])
    for k in range(KT):
        for m in range(8):
            nc.tensor.matmul(
                ps8[m], w1_tiles[m][:, ds(k * 128, 128)], h2t[k],
                start=(k == 0), stop=(k == KT - 1 and not has_bias["fc1"]))
    for m in range(8):
        if has_bias["fc1"]:
            nc.tensor.matmul(ps8[m], bias_sb[:, ds(b1_of + m * 128, 128)],
                             ones_r16, start=False, stop=True)
        g = p_g.tile([128, NQ], BF16, tag="g")
        nc.scalar.activation(g, ps8[m], gelu_func)
        g_sb.append(g)
    ps_m8.release()
    ps_m = tc.alloc_tile_pool(name="ps_m", bufs=4, space="PSUM")
    for m in range(8, MT1):
        if m in w1_tiles and w1_tiles[m] is not None:
            w1t = w1_tiles[m]
        else:
            w1t = p_w1.tile([128, C], BF16, tag="w1")
            nc.sync.dma_start(w1t, w1_m[m, :, :])
        ps = ps_m.tile([128, 512], F32, tag="ps_m")
        for k in range(KT):
            nc.tensor.matmul(
                ps, w1t[:, ds(k * 128, 128)], h2t[k],
                start=(k == 0), stop=(k == KT - 1 and not has_bias["fc1"]))
        if has_bias["fc1"]:
            nc.tensor.matmul(ps, bias_sb[:, ds(b1_of + m * 128, 128)],
                             ones_r16, start=False, stop=True)
        g = p_g.tile([128, NQ], BF16, tag="g")
        nc.scalar.activation(g, ps, gelu_func)
        g_sb.append(g)
    p_w1.release()

    for m in range(KT):
        if m in w2_tiles:
            w2t = w2_tiles[m]
        else:
            w2t = p_w2.tile([128, HID], BF16, tag="w2")
            nc.sync.dma_start(w2t, w2_m[m, :, :])
        ps = ps_m.tile([128, 512], F32, tag="ps_m")
        for k in range(HID // 128):
            nc.tensor.matmul(
                ps, w2t[:, ds(k * 128, 128)], g_sb[k],
                start=(k == 0), stop=(k == HID // 128 - 1 and not has_bias["fc2"]))
        if has_bias["fc2"]:
            nc.tensor.matmul(ps, bias_sb[:, ds(b2_of + m * 128, 128)],
                             ones_r16, start=False, stop=True)
        y = p_y.tile([128, NQ], F32, tag="y")
        nc.vector.tensor_add(y, ps, x2[m])
        nc.sync.dma_start(yT[ds(m * 128, 128), :], y)

    for p in (p_w2, p_y, p_g, p_h2, p_ln2, p_x2, pers):
        p.release()
    ps_m.release()
    p_dram.release()


# --------------------------------------------------------------------------
# Host side
# --------------------------------------------------------------------------
def _m_slice(w, mtiles):
    """[K_in, M_out] -> [mtiles, 128, K_in] with free dim k-major
    (arr[m, i, k*128+j] = w[k*128+i, m*128+j])."""
    kin = w.shape[0]
    kt = kin // 128
    a = w.reshape(kt, 128, mtiles, 128)        # [k, i, m, j]
    return np.ascontiguousarray(a.transpose(2, 1, 0, 3).reshape(mtiles, 128, kin))


def _prep(inputs):
    f32 = np.float32
    x = np.asarray(inputs["x"], f32)
    ln1_g = np.asarray(inputs["ln1_g"], f32)
    ln1_b = np.asarray(inputs["ln1_b"], f32)
    ln2_g = np.asarray(inputs["ln2_g"], f32)
    ln2_b = np.asarray(inputs["ln2_b"], f32)
    w_qkv = np.asarray(inputs["w_qkv"], f32)
    w_proj = np.asarray(inputs["w_proj"], f32)
    w_fc1 = np.asarray(inputs["w_fc1"], f32)
    w_fc2 = np.asarray(inputs["w_fc2"], f32)

    # fold LN affine params into the following matmul
    wqkv_e = ln1_g[:, None] * w_qkv
    bqkv_e = ln1_b @ w_qkv + np.asarray(inputs["b_qkv"], f32)
    wfc1_e = ln2_g[:, None] * w_fc1
    bfc1_e = ln2_b @ w_fc1 + np.asarray(inputs["b_fc1"], f32)
    b_proj = np.asarray(inputs["b_proj"], f32)
    b_fc2 = np.asarray(inputs["b_fc2"], f32)

    bf = ml_dtypes.bfloat16
    wq, wk, wvv = wqkv_e[:, :C], wqkv_e[:, C:2 * C], wqkv_e[:, 2 * C:]
    shared = {
        "wq_m": _m_slice(wq, KT).astype(bf),
        "wk_m": _m_slice(wk, KT).astype(bf),
        "wv_r": np.ascontiguousarray(wvv.reshape(KT, 128, C)).astype(bf),
        "wp_m": _m_slice(w_proj, KT).astype(bf),
        "w1_m": _m_slice(wfc1_e, HID // 128).astype(bf),
        "w2_m": _m_slice(w_fc2, KT).astype(bf),
        "b_all": np.concatenate(
            [bqkv_e, b_proj, bfc1_e, b_fc2])[None, :].astype(bf),
    }
    has_bias = {
        "qk": bool(np.any(bqkv_e[:2 * C])),
        "v": bool(np.any(bqkv_e[2 * C:])),
        "proj": bool(np.any(b_proj)),
        "fc1": bool(np.any(bfc1_e)),
        "fc2": bool(np.any(b_fc2)),
    }

    in_maps = []
    for c in range(8):
        b, half = c // 2, c % 2
        xb = x[b]
        if half:
            xb = np.concatenate([xb[NQ:], xb[:NQ]], axis=0)
        xt = np.ascontiguousarray(xb.T)
        m = {"xT": np.ascontiguousarray(xt[:, :NQ]), "xbT": xt.astype(bf),
             **shared}
        in_maps.append(m)
    return in_maps, has_bias


def kernel(**inputs):
    in_maps, has_bias = _prep(inputs)
    key = tuple(sorted(has_bias.items()))
    if key not in _cache:
        nc = build_program(has_bias)
        _split_wide_waits(nc, 1)
        _cache[key] = nc
    nc = _cache[key]

    res = bass_utils.run_bass_kernel_spmd(
        nc, in_maps, core_ids=list(range(8)), trace=False)

    x = np.asarray(inputs["x"])
    out = np.empty((4, NTOK, C), dtype=np.float32)
    for c in range(8):
        b, half = c // 2, c % 2
        out[b, half * NQ:(half + 1) * NQ, :] = res.results[c]["yT"].T
    return out.astype(x.dtype, copy=False)



# revision 11
# speedup vs baseline: 1.1039x; 1.1039x over previous
"""Trainium2 Bass kernel for a pre-norm transformer block (nn_Block).

Math (per batch b of x [4, 1024, 1024]):
    h  = LN(x) ; qkv = h @ w_qkv + b_qkv ; attention (16 heads, dh=64)
    x  = x + (attn_out @ w_proj + b_proj)
    h  = LN(x) ; x = x + gelu(h @ w_fc1 + b_fc1) @ w_fc2 + b_fc2

Sharding: communication-free hybrid. Core c handles batch b = c // 2 and
query-token half c % 2. Each core computes K and V for its batch's full
1024 tokens and everything else only for its own 512 query tokens.

Precision split (validated vs the fp32 reference in numpy):
  - qkv production, S^T, exp(P), V, PV run in fp8-e4m3 with DoubleRow
    perf mode (2 contraction blocks per matmul). Softmax's averaging over
    ~1024 keys washes the quantization noise out (rel_l2 ~1.7e-3).
  - proj and the MLP stay bf16 -- fp8 there pushes rel_l2 past 2e-2.

Layout is feature-major: activations live as [features, tokens]. fp8
weights are host-prepped into DoubleRow pair layout [p, i, f] where the
contraction index is c = (2j+i)*128+p. LN stats come from ones-vector
matmuls; per-token scale/shift rows are replicated across partitions with
gpsimd partition_broadcast (no DRAM bounce). Softmax denominators ride an
extra ones-column through the PV matmul (row 64 of the [65,512] psum) and
normalization is a single DVE divide against a partition_broadcast of that
row.
"""

import os
import sys

import numpy as np

try:
    import concourse.bass as bass
except ImportError:  # pragma: no cover
    for _p in ("/opt/trn_rl_repo", "/root/.axon_site/_ro/trn_rl_repo"):
        if os.path.isdir(_p) and _p not in sys.path:
            sys.path.insert(0, _p)
    import concourse.bass as bass

import ml_dtypes
import concourse.tile as tile
import concourse.mybir as mybir
from concourse import bass_utils
from concourse import library_config
from concourse.bass import ds

F32 = mybir.dt.float32
BF16 = mybir.dt.bfloat16
FP8 = mybir.dt.float8e4
AF = mybir.ActivationFunctionType
DR = mybir.MatmulPerfMode.DoubleRow

C = 1024          # model dim
H = 16            # heads
DH = 64           # head dim
NTOK = 1024       # tokens per batch (keys/values)
NQ = 512          # query tokens per core
KT = C // 128     # 8 feature tiles
JT = KT // 2      # 4 feature-pair tiles (DoubleRow)
HID = 4096
EPS = 1e-5
WSCALE = 2048.0   # pow2 scale folded into fp8 qkv weights
QSM = 1.0 / (WSCALE * float(DH) ** 0.5)   # Q psum -> fp8 cast scale
KSM = 1.0 / WSCALE                        # K/V psum -> fp8 cast scale

# this image's walrus cannot compile the gpsimd custom-ISA ops
# (partition_broadcast) or custom DVE ops; broadcasts go through a DRAM
# bounce + to_broadcast DMA instead.
USE_GPSIMD_BCAST = False

_cache = {}


def _split_wide_waits(nc, max_waits=1):
    """Walrus on this image rejects instructions carrying more than one
    semaphore wait; split the excess onto same-engine NOPs."""
    ctr = 0
    for f in nc.m.functions:
        for b in f.blocks:
            out, changed = [], False
            for inst in b.instructions:
                si = getattr(inst, "sync_info", None)
                if si is not None and si.on_wait and len(si.on_wait) > max_waits:
                    waits = list(si.on_wait)
                    extra, keep = waits[:-max_waits], waits[-max_waits:]
                    for gs in range(0, len(extra), max_waits):
                        ctr += 1
                        nop = mybir.InstNoOp(
                            name=f"waitsplit-{ctr}", ins=[], outs=[])
                        nop.engine = inst.engine
                        nop.sync_info = mybir.SyncInfo(
                            on_wait=extra[gs:gs + max_waits], on_update=[])
                        out.append(nop)
                    inst.sync_info = mybir.SyncInfo(
                        on_wait=keep, on_update=list(si.on_update))
                    changed = True
                out.append(inst)
            if changed:
                b.instructions = out


def build_program(has_bias, gelu_func=None):
    nc = bass.Bass()

    xT = nc.dram_tensor("xT", [C, NQ], F32, kind="ExternalInput")
    xbT = nc.dram_tensor("xbT", [C, NTOK], BF16, kind="ExternalInput")
    # fp8 DoubleRow pair layouts (see _prep)
    wq8 = nc.dram_tensor("wq8", [KT, 128, C], FP8, kind="ExternalInput")
    wk8 = nc.dram_tensor("wk8", [KT, 128, C], FP8, kind="ExternalInput")
    wv8 = nc.dram_tensor("wv8", [JT, 128, 2 * C], FP8, kind="ExternalInput")
    wp_m = nc.dram_tensor("wp_m", [KT, 128, C], BF16, kind="ExternalInput")
    w1_m = nc.dram_tensor("w1_m", [HID // 128, 128, C], BF16, kind="ExternalInput")
    w2_m = nc.dram_tensor("w2_m", [KT, 128, HID], BF16, kind="ExternalInput")
    b_all = nc.dram_tensor("b_all", [1, 3 * C + C + HID + C], BF16,
                           kind="ExternalInput")
    yT = nc.dram_tensor("yT", [C, NQ], F32, kind="ExternalOutput")

    with tile.TileContext(nc) as tc:
        _emit(nc, tc, xT, xbT, wq8, wk8, wv8, wp_m, w1_m, w2_m, b_all,
              yT, has_bias, gelu_func or AF.Gelu)
    return nc


def _emit(nc, tc, xT, xbT, wq8, wk8, wv8, wp_m, w1_m, w2_m, b_all, yT,
          has_bias, gelu_func):
    pers = tc.alloc_tile_pool(name="pers", bufs=1)
    ones_c = pers.tile([128, 1], BF16, tag="ones_c")      # stats lhsT
    nc.vector.memset(ones_c, 1.0)
    ones_r16 = pers.tile([1, NQ], BF16, tag="ones_r16")   # bias rank-1 rhs
    nc.vector.memset(ones_r16, 1.0)
    ones_tok16 = pers.tile([1, 128], BF16, tag="ones_tok16")  # v-bias lhsT
    nc.vector.memset(ones_tok16, 1.0)
    eps_t = pers.tile([128, 1], F32, tag="eps_t")
    nc.vector.memset(eps_t, EPS)

    p_dram = tc.alloc_tile_pool(name="dscratch", bufs=4, space="DRAM")

    def ln_chain(ms, ss, N, pool, nm):
        """From per-chunk sum/sumsq PSUM rows, produce [128, N] rstd_rep and
        (mu*rstd)_rep. Row math stays [1, N]; the partition replication is a
        single gpsimd partition_broadcast (or a DRAM bounce fallback)."""
        row = pool.tile([1, 2 * N], F32, tag=f"row_{nm}", name=f"row_{nm}")
        nch = N // 512
        for n in range(nch):
            nc.vector.tensor_copy(row[:, ds(n * 512, 512)], ms[n])
            nc.vector.tensor_copy(row[:, ds(N + n * 512, 512)], ss[n])
        mu = row[:, ds(0, N)]
        es = row[:, ds(N, N)]
        nc.vector.tensor_scalar_mul(mu, mu, 1.0 / C)
        nc.vector.tensor_scalar_mul(es, es, 1.0 / C)
        var = pool.tile([1, N], F32, tag=f"var_{nm}", name=f"var_{nm}")
        nc.vector.tensor_mul(var, mu, mu)
        nc.vector.tensor_sub(var, es, var)
        lnv = pool.tile([1, N], F32, tag=f"lnv_{nm}", name=f"lnv_{nm}")
        nc.scalar.activation(lnv, var, AF.Ln, bias=eps_t[ds(0, 1), :])
        out2 = pool.tile([1, 2 * N], F32, tag=f"out2_{nm}", name=f"out2_{nm}")
        rstd = out2[:, ds(0, N)]
        nc.scalar.activation(rstd, lnv, AF.Exp, scale=-0.5)
        nc.vector.tensor_mul(out2[:, ds(N, N)], mu, rstd)
        rep = pool.tile([128, 2 * N], F32, tag=f"rep_{nm}", name=f"rep_{nm}")
        if USE_GPSIMD_BCAST:
            nc.gpsimd.partition_broadcast(rep, out2)
        else:
            drow = p_dram.tile([1, 2 * N], F32, tag="dscratch", name=f"dr_{nm}")
            nc.gpsimd.dma_start(drow, out2)
            for q in range(4):
                nc.gpsimd.dma_start(rep[ds(q * 32, 32), :],
                                    drow.to_broadcast((32, 2 * N)))
        return rep[:, ds(0, N)], rep[:, ds(N, N)]

    any_bias = any(has_bias.values())
    if any_bias:
        bias_sb = pers.tile([1, 3 * C + C + HID + C], BF16, tag="bias_sb")
        nc.sync.dma_start(bias_sb, b_all[:])
        bq_of, bk_of, bv_of = 0, C, 2 * C
        bp_of, b1_of, b2_of = 3 * C, 4 * C, 4 * C + HID

    # x2 (attention residual) is read until program end
    p_x2 = tc.alloc_tile_pool(name="x2", bufs=KT)
    p_xT = tc.alloc_tile_pool(name="xT", bufs=KT)
    xt = []

    p_V = tc.alloc_tile_pool(name="V", bufs=JT)
    p_wv = tc.alloc_tile_pool(name="wv", bufs=JT)
    p_h1 = tc.alloc_tile_pool(name="h1", bufs=JT)
    p_xb = tc.alloc_tile_pool(name="xb", bufs=KT)
    p_ln1 = tc.alloc_tile_pool(name="ln1", bufs=1)
    p_sq = tc.alloc_tile_pool(name="sq", bufs=3)
    ps_stat = tc.alloc_tile_pool(name="ps_stat", bufs=1, space="PSUM")

    xbt = []
    for k in range(KT):
        t = p_xb.tile([128, NTOK], BF16, tag="xb")
        nc.sync.dma_start(t, xbT[ds(k * 128, 128), :])
        xbt.append(t)
    # V weights early so V matmuls are never DMA-gated
    wv = []
    for j in range(JT):
        t = p_wv.tile([128, 2 * C], FP8, tag="wv")
        nc.sync.dma_start(t, wv8[j, :, :])
        wv.append(t)
    # V2[r]: pair layout [128 tok, i(2), H, 65] fp8; [.., 64] is the ones col
    V2 = []
    for r in range(JT):
        vt = p_V.tile([128, 2, H, 65], FP8, tag="V", name=f"V{r}")
        nc.vector.memset(vt[:, :, :, ds(64, 1)], 1.0)
        V2.append(vt)

    ms = [ps_stat.tile([1, 512], F32, tag=f"ms{n}", name=f"ms{n}")
          for n in range(2)]
    ss = [ps_stat.tile([1, 512], F32, tag=f"ss{n}", name=f"ss{n}")
          for n in range(2)]
    for k in range(KT):
        sq = p_sq.tile([128, NTOK], BF16, tag="sq")
        nc.gpsimd.tensor_mul(sq, xbt[k], xbt[k])
        for n in range(2):
            nc.tensor.matmul(ms[n], ones_c, xbt[k][:, ds(n * 512, 512)],
                             start=(k == 0), stop=(k == KT - 1))
            nc.tensor.matmul(ss[n], ones_c, sq[:, ds(n * 512, 512)],
                             start=(k == 0), stop=(k == KT - 1))
    p_sq.release()

    rstd_rep, musc_rep = ln_chain(ms, ss, NTOK, p_ln1, "ln1")
    ps_stat.release()

    # h1p[j]: fp8 pair tile [128, 2, NTOK]; halves are feature blocks 2j, 2j+1
    p_tmp = tc.alloc_tile_pool(name="tmp", bufs=4)
    h1 = []
    for j in range(JT):
        h = p_h1.tile([128, 2, NTOK], FP8, tag="h1", name=f"h1p{j}")
        h1.append(h)
    for k in range(KT):
        tmp = p_tmp.tile([128, NTOK], F32, tag="tmp")
        nc.vector.tensor_mul(tmp, xbt[k], rstd_rep)
        nc.vector.tensor_sub(h1[k // 2][:, k % 2, :], tmp, musc_rep)
    p_tmp.release()
    p_ln1.release()
    p_xb.release()

    # ---------------- K/Q psum pool first so its 2 banks are reserved ----
    ps_kq = tc.alloc_tile_pool(name="ps_kq", bufs=2, space="PSUM")
    ps_v = tc.alloc_tile_pool(name="ps_v", bufs=6, space="PSUM")

    # ---------------- V (token-major, DoubleRow over feature pairs) -----
    # groups of <=3 token-tiles so V casts of group g overlap matmuls of g+1
    for g0 in range(0, KT, 3):
        ts_ = range(g0, min(g0 + 3, KT))
        psv = {(t, n): ps_v.tile([128, 512], F32, tag="ps_v",
                                 name=f"psv{t}_{n}")
               for t in ts_ for n in range(2)}
        for j in range(JT):
            for t in ts_:
                for n in range(2):
                    nc.tensor.matmul(
                        psv[(t, n)], h1[j][:, :, ds(t * 128, 128)],
                        wv[j].rearrange("p (i f) -> p i f", i=2)[
                            :, :, ds(n * 512, 512)],
                        start=(j == 0),
                        stop=(j == JT - 1 and not has_bias["v"]),
                        perf_mode=DR)
        for t in ts_:
            for n in range(2):
                if has_bias["v"]:
                    nc.tensor.matmul(
                        psv[(t, n)], ones_tok16,
                        bias_sb[:, ds(bv_of + n * 512, 512)],
                        start=False, stop=True)
                nc.vector.tensor_scalar_mul(
                    V2[t // 2][:, t % 2, ds(n * 8, 8), ds(0, 64)],
                    psv[(t, n)].rearrange("p (h d) -> p h d", d=64), KSM)
    ps_v.release()

    # ---------------- attention loop over head pairs --------------------
    p_O = tc.alloc_tile_pool(name="O", bufs=KT)
    p_K = tc.alloc_tile_pool(name="K", bufs=KT)
    p_Q = tc.alloc_tile_pool(name="Q", bufs=KT)
    p_P = tc.alloc_tile_pool(name="P", bufs=18)
    p_rq = tc.alloc_tile_pool(name="rq", bufs=4)
    p_rep = tc.alloc_tile_pool(name="rep", bufs=4)
    p_wkq = tc.alloc_tile_pool(name="wkq", bufs=4)
    ps_s = tc.alloc_tile_pool(name="ps_s", bufs=2, space="PSUM")
    ps_o = tc.alloc_tile_pool(name="ps_o", bufs=2, space="PSUM")

    K_sb, Q_sb, P_sb, O_sb = [], [], {}, []

    def emit_kq(t):
        wkt = p_wkq.tile([128, C], FP8, tag="wkq")
        nc.sync.dma_start(wkt, wk8[t, :, :])
        kt_sb = p_K.tile([128, NTOK], FP8, tag="K")
        wkp = wkt.rearrange("p (j i f) -> p j i f", j=JT, i=2)
        for n in range(2):
            ps = ps_kq.tile([128, 512], F32, tag="ps_kq")
            for j in range(JT):
                nc.tensor.matmul(
                    ps, wkp[:, j],
                    h1[j][:, :, ds(n * 512, 512)],
                    start=(j == 0), stop=(j == JT - 1 and not has_bias["qk"]),
                    perf_mode=DR)
            if has_bias["qk"]:
                nc.tensor.matmul(
                    ps, bias_sb[:, ds(bk_of + t * 128, 128)], ones_r16,
                    start=False, stop=True)
            nc.vector.tensor_scalar_mul(kt_sb[:, ds(n * 512, 512)], ps, KSM)
        K_sb.append(kt_sb)

        wqt = p_wkq.tile([128, C], FP8, tag="wkq")
        nc.sync.dma_start(wqt, wq8[t, :, :])
        qt_sb = p_Q.tile([128, NQ], FP8, tag="Q")
        wqp = wqt.rearrange("p (j i f) -> p j i f", j=JT, i=2)
        ps = ps_kq.tile([128, 512], F32, tag="ps_kq")
        for j in range(JT):
            nc.tensor.matmul(
                ps, wqp[:, j], h1[j][:, :, ds(0, 512)],
                start=(j == 0), stop=(j == JT - 1 and not has_bias["qk"]),
                perf_mode=DR)
        if has_bias["qk"]:
            nc.tensor.matmul(
                ps, bias_sb[:, ds(bq_of + t * 128, 128)], ones_r16,
                start=False, stop=True)
        nc.vector.tensor_scalar_mul(qt_sb, ps, QSM)
        Q_sb.append(qt_sb)

    def emit_st(t):
        # S^T per key-block pair r: [128, 1024] psum (2 banks), both heads;
        # one exp per (h2, r) -> P pair tile = PV DoubleRow moving operand
        for h2 in range(2):
            lo = h2 * 64
            for r in range(JT):
                ps = ps_s.tile([128, 1024], F32, tag="ps_s")
                for i in range(2):
                    m = 2 * r + i
                    nc.tensor.matmul(
                        ps[:, ds(i * 512, 512)],
                        K_sb[t][ds(lo, 64), ds(m * 128, 128)],
                        Q_sb[t][ds(lo, 64), :],
                        start=True, stop=True)
                p = p_P.tile([128, 2, 512], FP8, tag="P")
                nc.scalar.activation(
                    p.rearrange("p i f -> p (i f)"), ps, AF.Exp)
                P_sb[(t, h2, r)] = p

    def emit_pv(t):
        # PV with the ones-column denominator in psum row 64. Both heads'
        # den rows pack into one [2,512] tile so a single Ln+Exp pair on ACT
        # computes both reciprocals; psum is evacuated to O (bf16) right away
        # so the bank frees before the bounce-DMA latency, then O is scaled
        # in place.
        ot = p_O.tile([128, NQ], BF16, tag="O")
        drr = p_dram.tile([2, 512], F32, tag="dscratch", name=f"drr{t}")
        rep = p_rep.tile([128, 512], F32, tag="rep")
        for h2 in range(2):
            head = 2 * t + h2
            ps = ps_o.tile([65, 512], F32, tag="ps_o")
            for r in range(JT):
                nc.tensor.matmul(
                    ps, V2[r][:, :, head, :], P_sb[(t, h2, r)],
                    start=(r == 0), stop=(r == JT - 1),
                    perf_mode=DR)
            lnr = p_rq.tile([1, 512], F32, tag="lnr")
            nc.scalar.activation(lnr, ps[ds(64, 1), :], AF.Ln)
            nc.vector.tensor_copy(ot[ds(h2 * 64, 64), :], ps[ds(0, 64), :])
            rcp = p_rq.tile([1, 512], F32, tag="rcp")
            nc.scalar.activation(rcp, lnr, AF.Exp, scale=-1.0)
            nc.gpsimd.dma_start(drr[ds(h2, 1), :], rcp)
            nc.gpsimd.dma_start(rep[ds(h2 * 64, 64), :],
                                drr[ds(h2, 1), :].to_broadcast((64, 512)))
        nc.vector.tensor_mul(ot, ot, rep)
        O_sb.append(ot)

    for t in range(KT):
        emit_kq(t)
        xq = p_xT.tile([128, NQ], F32, tag="xT", name=f"xq{t}")
        nc.sync.dma_start(xq, xT[ds(t * 128, 128), :])
        xt.append(xq)
        emit_st(t)
        if t >= 1:
            emit_pv(t - 1)
    emit_pv(KT - 1)
    for p in (p_wkq, p_rep, p_rq, p_P, p_Q, p_K):
        p.release()
    for p in (ps_o, ps_s, ps_kq):
        p.release()

    # ---------------- proj + residual + LN2 stats ----------------
    p_wp = tc.alloc_tile_pool(name="wp", bufs=3)
    p_sq2 = tc.alloc_tile_pool(name="sq2", bufs=3)
    ps_st2 = tc.alloc_tile_pool(name="ps_st2", bufs=1, space="PSUM")
    ps_p = tc.alloc_tile_pool(name="ps_p", bufs=3, space="PSUM")

    ms2 = ps_st2.tile([1, 512], F32, tag="ms2")
    ss2 = ps_st2.tile([1, 512], F32, tag="ss2")
    x2 = []
    for m in range(KT):
        wpt = p_wp.tile([128, C], BF16, tag="wp")
        nc.sync.dma_start(wpt, wp_m[m, :, :])
        ps = ps_p.tile([128, 512], F32, tag="ps_p")
        for k in range(KT):
            nc.tensor.matmul(
                ps, wpt[:, ds(k * 128, 128)], O_sb[k],
                start=(k == 0), stop=(k == KT - 1 and not has_bias["proj"]))
        if has_bias["proj"]:
            nc.tensor.matmul(ps, bias_sb[:, ds(bp_of + m * 128, 128)],
                             ones_r16, start=False, stop=True)
        xm = p_x2.tile([128, NQ], F32, tag="x2")
        nc.vector.tensor_add(xm, ps, xt[m])
        x2.append(xm)
        xb2 = p_sq2.tile([128, NQ], BF16, tag="xb2")
        nc.vector.tensor_copy(xb2, xm)
        sq = p_sq2.tile([128, NQ], BF16, tag="sq2")
        nc.gpsimd.tensor_mul(sq, xb2, xb2)
        nc.tensor.matmul(ms2, ones_c, xb2,
                         start=(m == 0), stop=(m == KT - 1))
        nc.tensor.matmul(ss2, ones_c, sq,
                         start=(m == 0), stop=(m == KT - 1))

    for p in (p_sq2, p_wp, p_O, p_h1, p_wv, p_V, p_xT):
        p.release()
    ps_p.release()

    # ---------------- LN2 ----------------
    p_ln2 = tc.alloc_tile_pool(name="ln2", bufs=1)
    rstd2_rep, musc2_rep = ln_chain([ms2], [ss2], NQ, p_ln2, "ln2")
    ps_st2.release()

    p_h2 = tc.alloc_tile_pool(name="h2", bufs=KT)
    p_tmp2 = tc.alloc_tile_pool(name="tmp2", bufs=6)
    h2t = []
    for k in range(KT):
        tmp = p_tmp2.tile([128, NQ], F32, tag="tmp2")
        nc.vector.tensor_mul(tmp, x2[k], rstd2_rep)
        h = p_h2.tile([128, NQ], BF16, tag="h2")
        nc.vector.tensor_sub(h, tmp, musc2_rep)
        h2t.append(h)
    p_tmp2.release()

    # ---------------- MLP (bf16) ----------------
    p_g = tc.alloc_tile_pool(name="g", bufs=HID // 128)
    p_y = tc.alloc_tile_pool(name="y", bufs=3)
    p_w2 = tc.alloc_tile_pool(name="w2", bufs=3)
    p_w1 = tc.alloc_tile_pool(name="w1", bufs=9)
    ps_m8 = tc.alloc_tile_pool(name="ps_m8", bufs=8, space="PSUM")

    w1_tiles, w2_tiles = {}, {}
    for m in range(3):
        w1_tiles[m] = p_w1.tile([128, C], BF16, tag="w1", name=f"w1p{m}")
        nc.sync.dma_start(w1_tiles[m], w1_m[m, :, :])
    for m in range(2):
        w2_tiles[m] = p_w2.tile([128, HID], BF16, tag="w2", name=f"w2p{m}")
        nc.sync.dma_start(w2_tiles[m], w2_m[m, :, :])

    g_sb = []
    MT1 = HID // 128
    ps8 = [ps_m8.tile([128, 512], F32, tag="ps8", name=f"ps8_{m}")
           for m in range(8)]
    for m in range(8):
        if w1_tiles.get(m) is None:
            w1_tiles[m] = p_w1.tile([128, C], BF16, tag="w1", name=f"w1g{m}")
            nc.sync.dma_start(w1_tiles[m], w1_m[m, :, :])
    for k in range(KT):
        for m in range(8):
            nc.tensor.matmul(
                ps8[m], w1_tiles[m][:, ds(k * 128, 128)], h2t[k],
                start=(k == 0), stop=(k == KT - 1 and not has_bias["fc1"]))
    for m in range(8):
        if has_bias["fc1"]:
            nc.tensor.matmul(ps8[m], bias_sb[:, ds(b1_of + m * 128, 128)],
                             ones_r16, start=False, stop=True)
        g = p_g.tile([128, NQ], BF16, tag="g")
        nc.scalar.activation(g, ps8[m], gelu_func)
        g_sb.append(g)
    ps_m8.release()
    ps_m = tc.alloc_tile_pool(name="ps_m", bufs=4, space="PSUM")
    for m in range(8, MT1):
        if w1_tiles.get(m) is not None:
            w1t = w1_tiles[m]
        else:
            w1t = p_w1.tile([128, C], BF16, tag="w1")
            nc.sync.dma_start(w1t, w1_m[m, :, :])
        ps = ps_m.tile([128, 512], F32, tag="ps_m")
        for k in range(KT):
            nc.tensor.matmul(
                ps, w1t[:, ds(k * 128, 128)], h2t[k],
                start=(k == 0), stop=(k == KT - 1 and not has_bias["fc1"]))
        if has_bias["fc1"]:
            nc.tensor.matmul(ps, bias_sb[:, ds(b1_of + m * 128, 128)],
                             ones_r16, start=False, stop=True)
        g = p_g.tile([128, NQ], BF16, tag="g")
        nc.scalar.activation(g, ps, gelu_func)
        g_sb.append(g)
    p_w1.release()

    for m in range(KT):
        if m in w2_tiles:
            w2t = w2_tiles[m]
        else:
            w2t = p_w2.tile([128, HID], BF16, tag="w2")
            nc.sync.dma_start(w2t, w2_m[m, :, :])
        ps = ps_m.tile([128, 512], F32, tag="ps_m")
        for k in range(HID // 128):
            nc.tensor.matmul(
                ps, w2t[:, ds(k * 128, 128)], g_sb[k],
                start=(k == 0), stop=(k == HID // 128 - 1 and not has_bias["fc2"]))
        if has_bias["fc2"]:
            nc.tensor.matmul(ps, bias_sb[:, ds(b2_of + m * 128, 128)],
                             ones_r16, start=False, stop=True)
        y = p_y.tile([128, NQ], F32, tag="y")
        nc.vector.tensor_add(y, ps, x2[m])
        nc.sync.dma_start(yT[ds(m * 128, 128), :], y)

    for p in (p_w2, p_y, p_g, p_h2, p_ln2, p_x2, pers):
        p.release()
    ps_m.release()
    p_dram.release()


# --------------------------------------------------------------------------
# Host side
# --------------------------------------------------------------------------
def _m_slice(w, mtiles):
    """[K_in, M_out] -> [mtiles, 128, K_in] with free dim k-major."""
    kin = w.shape[0]
    kt = kin // 128
    a = w.reshape(kt, 128, mtiles, 128)        # [k, i, m, j]
    return np.ascontiguousarray(a.transpose(2, 1, 0, 3).reshape(mtiles, 128, kin))


def _pair_m(w, mtiles):
    """fp8 DoubleRow stationary layout for out^T = w^T @ act:
    [K_in, M_out] -> [mtiles, 128, JT*2*128]: out[t, p, j*256 + i*128 + f]
    = w[(2j+i)*128 + p, t*128 + f]."""
    kin = w.shape[0]
    jt = kin // 256
    a = w.reshape(jt, 2, 128, mtiles, 128)     # [j, i, p, t, f]
    return np.ascontiguousarray(
        a.transpose(3, 2, 0, 1, 4).reshape(mtiles, 128, jt * 256))


def _pair_r(w):
    """fp8 DoubleRow moving layout for token-major V = act^T @ w:
    [K_in, F_out] -> [JT, 128, 2*F]: out[j, p, i*F + f] = w[(2j+i)*128+p, f]."""
    kin, f = w.shape
    jt = kin // 256
    a = w.reshape(jt, 2, 128, f)               # [j, i, p, f]
    return np.ascontiguousarray(a.transpose(0, 2, 1, 3).reshape(jt, 128, 2 * f))


def _prep(inputs):
    f32 = np.float32
    x = np.asarray(inputs["x"], f32)
    ln1_g = np.asarray(inputs["ln1_g"], f32)
    ln1_b = np.asarray(inputs["ln1_b"], f32)
    ln2_g = np.asarray(inputs["ln2_g"], f32)
    ln2_b = np.asarray(inputs["ln2_b"], f32)
    w_qkv = np.asarray(inputs["w_qkv"], f32)
    w_proj = np.asarray(inputs["w_proj"], f32)
    w_fc1 = np.asarray(inputs["w_fc1"], f32)
    w_fc2 = np.asarray(inputs["w_fc2"], f32)

    wqkv_e = ln1_g[:, None] * w_qkv
    bqkv_e = ln1_b @ w_qkv + np.asarray(inputs["b_qkv"], f32)
    wfc1_e = ln2_g[:, None] * w_fc1
    bfc1_e = ln2_b @ w_fc1 + np.asarray(inputs["b_fc1"], f32)
    b_proj = np.asarray(inputs["b_proj"], f32)
    b_fc2 = np.asarray(inputs["b_fc2"], f32)

    bf = ml_dtypes.bfloat16
    f8 = ml_dtypes.float8_e4m3
    wq, wk, wvv = wqkv_e[:, :C], wqkv_e[:, C:2 * C], wqkv_e[:, 2 * C:]

    def q8(a):
        return np.clip(a * WSCALE, -240, 240).astype(f8)

    shared = {
        "wq8": q8(_pair_m(wq, KT)),
        "wk8": q8(_pair_m(wk, KT)),
        "wv8": q8(_pair_r(wvv)),
        "wp_m": _m_slice(w_proj, KT).astype(bf),
        "w1_m": _m_slice(wfc1_e, HID // 128).astype(bf),
        "w2_m": _m_slice(w_fc2, KT).astype(bf),
        # qkv biases ride the fp8-scaled psum, so pre-scale them by WSCALE
        "b_all": np.concatenate(
            [bqkv_e * WSCALE, b_proj, bfc1_e, b_fc2])[None, :].astype(bf),
    }
    has_bias = {
        "qk": bool(np.any(bqkv_e[:2 * C])),
        "v": bool(np.any(bqkv_e[2 * C:])),
        "proj": bool(np.any(b_proj)),
        "fc1": bool(np.any(bfc1_e)),
        "fc2": bool(np.any(b_fc2)),
    }

    in_maps = []
    for c in range(8):
        b, half = c // 2, c % 2
        xb = x[b]
        if half:
            xb = np.concatenate([xb[NQ:], xb[:NQ]], axis=0)
        xt = np.ascontiguousarray(xb.T)
        m = {"xT": np.ascontiguousarray(xt[:, :NQ]), "xbT": xt.astype(bf),
             **shared}
        in_maps.append(m)
    return in_maps, has_bias


def kernel(**inputs):
    in_maps, has_bias = _prep(inputs)
    key = tuple(sorted(has_bias.items()))
    if key not in _cache:
        nc = build_program(has_bias)
        _split_wide_waits(nc, 1)
        _cache[key] = nc
    nc = _cache[key]

    res = bass_utils.run_bass_kernel_spmd(
        nc, in_maps, core_ids=list(range(8)), trace=False)

    x = np.asarray(inputs["x"])
    out = np.empty((4, NTOK, C), dtype=np.float32)
    for c in range(8):
        b, half = c // 2, c % 2
        out[b, half * NQ:(half + 1) * NQ, :] = res.results[c]["yT"].T
    return out.astype(x.dtype, copy=False)
